# revision 12
# baseline (speedup 1.0000x reference)
"""Trainium2 Bass kernel for nn_AMIPRouterInference (gnn_message_passing).

Strategy
--------
Algebraic restructure of the reference (~515 GFLOP -> ~52 GFLOP):
  * cond @ W1 splits into h_anc @ W1a + h_ctr @ W1b, each computed once per
    token (not once per window pair):  u = h @ W1b, v = h @ W1a.
  * The attention combine over the +-r window commutes with the W2 matmul:
    g = sum_n cw_n * gelu(v[l+off_n] + u[l]);  delta = (w * g) @ W2 + w @ b2.

Sharding: pure data-parallel over the B*L = 4096 tokens -> 512 tokens/core on
8 cores; the +-5 halo is baked into each core's input shard on the host, so no
collectives are needed.

Per-core layout: features-on-partitions (u/v as 16 chunks of [128, tokens]) so
window shifts along tokens are free-axis SBUF slices.  Even/odd phase copies of
v keep the bf16 DVE 2x alignment for shifted adds.
"""

import sys

for _p in ("/opt/trn_rl_repo", "/root/.axon_site/_ro/trn_rl_repo"):
    if _p not in sys.path:
        sys.path.append(_p)

import numpy as np
import ml_dtypes

import concourse.bacc as bacc
import concourse.mybir as mybir
import concourse.tile as tile
from concourse.bass_utils import run_bass_kernel_spmd

BF16 = ml_dtypes.bfloat16

# Problem constants (hardcoded per spec).
B, L, D = 2, 2048, 1024
K, D4, R = 8, 256, 5
NCORES = 8
T = (B * L) // NCORES          # tokens per core = 512
PADL = 16                      # left pad of the per-core token window
TP = T + 2 * PADL              # padded width = 544
NOFF = 2 * R                   # 10 window offsets
F = K * D4                     # 2048 fused expert features
NFC = F // 128                 # 16 feature chunks
NKC = D // 128                 # 8 contraction chunks
NTC = T // 128                 # 4 token tiles per core

# Offset processing order: even offsets first (read from v_even), then odd
# (read from v_odd, which holds v shifted left by one token).  Within each
# phase the SBUF slice starts are even element indices -> 4-byte aligned.
OFF_ORDER = [-4, -2, 2, 4, -5, -3, -1, 1, 3, 5]

_CACHE = {}


def _build_graph():
    fp32 = mybir.dt.float32
    bf16 = mybir.dt.bfloat16

    nc = bacc.Bacc("TRN2", target_bir_lowering=False, debug=False,
                   num_devices=NCORES)

    # ---- DRAM parameters (per-core shards; same shapes on every core) ----
    hT = nc.dram_tensor("hT", [NKC, 128, TP], bf16, kind="ExternalInput")
    w1a = nc.dram_tensor("w1a", [NFC, 128, D], bf16, kind="ExternalInput")
    w1b = nc.dram_tensor("w1b", [NFC, 128, D], bf16, kind="ExternalInput")
    w2 = nc.dram_tensor("w2", [NFC, 128, D], bf16, kind="ExternalInput")
    wr = nc.dram_tensor("wr", [NKC, 128, K], bf16, kind="ExternalInput")
    valid = nc.dram_tensor("valid", [NTC, 128, NOFF], fp32, kind="ExternalInput")
    negm = nc.dram_tensor("negm", [NTC, 128, NOFF], fp32, kind="ExternalInput")
    br_bc = nc.dram_tensor("br_bc", [128, K], fp32, kind="ExternalInput")
    b1s = nc.dram_tensor("b1s", [128, NFC], fp32, kind="ExternalInput")
    b2s = nc.dram_tensor("b2s", [K, D], bf16, kind="ExternalInput")
    ident = nc.dram_tensor("ident", [128, 128], fp32, kind="ExternalInput")
    ones = nc.dram_tensor("ones", [1, 128], bf16, kind="ExternalInput")
    out = nc.dram_tensor("out", [T, D], fp32, kind="ExternalOutput")

    AF = mybir.ActivationFunctionType
    OP = mybir.AluOpType

    with tile.TileContext(nc) as tc:
        with (
            tc.tile_pool(name="const", bufs=1) as cpool,
            tc.tile_pool(name="hpool", bufs=1) as hpool,
            tc.tile_pool(name="w2pool", bufs=1) as w2pool,
            tc.tile_pool(name="w1pool", bufs=3) as w1pool,
            tc.tile_pool(name="small", bufs=3) as spool,
            tc.tile_pool(name="persist", bufs=1) as ppool,
            tc.tile_pool(name="uv", bufs=2) as uvpool,
            tc.tile_pool(name="big", bufs=2) as bigpool,
            tc.tile_pool(name="qbuf", bufs=1) as q1pool,
            tc.tile_pool(name="tbuf", bufs=2) as qpool,
            tc.tile_pool(name="gout", bufs=1) as gpool,
            tc.tile_pool(name="opool", bufs=2) as opool,
            tc.tile_pool(name="ps_big", bufs=4, space="PSUM") as psb,
            tc.tile_pool(name="ps_small", bufs=4, space="PSUM") as pss,
            # NOTE: all psb tiles share tag "m" (4 banks), all pss tiles share
            # tag "s" (4 banks) -- PSUM has only 8 banks total.
        ):
            # ---------------- constant / input loads ----------------
            h_sb = []
            for kc in range(NKC):
                t = hpool.tile([128, TP], bf16, tag=f"h{kc}")
                nc.sync.dma_start(t[:], hT[kc])
                h_sb.append(t)
            wr_sb = []
            for kc in range(NKC):
                t = cpool.tile([128, K], bf16, tag=f"wr{kc}")
                nc.sync.dma_start(t[:], wr[kc])
                wr_sb.append(t)
            w2_sb = []
            for fc in range(NFC):
                t = w2pool.tile([128, D], bf16, tag=f"w2_{fc}")
                nc.sync.dma_start(t[:], w2[fc])
                w2_sb.append(t)
            ident_sb = cpool.tile([128, 128], fp32, tag="ident")
            nc.sync.dma_start(ident_sb[:], ident[:])
            ones_sb = cpool.tile([1, 128], bf16, tag="ones")
            nc.sync.dma_start(ones_sb[:], ones[:])
            br_sb = cpool.tile([128, K], fp32, tag="br")
            nc.sync.dma_start(br_sb[:], br_bc[:])
            b1_sb = cpool.tile([128, NFC], fp32, tag="b1")
            nc.sync.dma_start(b1_sb[:], b1s[:])
            b2_sb = cpool.tile([K, D], bf16, tag="b2")
            nc.sync.dma_start(b2_sb[:], b2s[:])
            valid_sb, neg_sb = [], []
            for tci in range(NTC):
                tv = cpool.tile([128, NOFF], fp32, tag=f"val{tci}")
                nc.sync.dma_start(tv[:], valid[tci])
                valid_sb.append(tv)
                tn = cpool.tile([128, NOFF], fp32, tag=f"neg{tci}")
                nc.sync.dma_start(tn[:], negm[tci])
                neg_sb.append(tn)

            # persistent transposed score & router weights (bf16)
            cwT_bf = ppool.tile([NOFF, T], bf16, tag="cwT")
            wT_bf = ppool.tile([K, T], bf16, tag="wT")

            # ------------- stage A/B/C: scores, cw, router w -------------
            for tci in range(NTC):
                c0 = PADL + tci * 128
                g_ps = psb.tile([128, 512], fp32, tag="m")
                lg_ps = pss.tile([128, K], fp32, tag="s")
                for kc in range(NKC):
                    st = (kc == 0)
                    sp = (kc == NKC - 1)
                    nc.tensor.matmul(g_ps[:, :138],
                                     h_sb[kc][:, c0:c0 + 128],
                                     h_sb[kc][:, c0 - 5:c0 + 133],
                                     start=st, stop=sp)
                    nc.tensor.matmul(lg_ps[:],
                                     h_sb[kc][:, c0:c0 + 128],
                                     wr_sb[kc][:],
                                     start=st, stop=sp)
                # scores: extract 10 shifted diagonals of the gram tile
                s_t = spool.tile([128, NOFF], fp32, tag="s")
                junk = spool.tile([128, 128], fp32, tag="junk")
                for n, off in enumerate(OFF_ORDER):
                    nc.vector.affine_mul_reduce(
                        junk[:], s_t[:, n:n + 1],
                        g_ps[:, off + 5:off + 5 + 128], ident_sb[:],
                        1.0 / 32.0, 0.0)
                # masked softmax over the offset axis
                sm = spool.tile([128, NOFF], fp32, tag="sm")
                nc.vector.tensor_mul(sm[:], s_t[:], valid_sb[tci][:])
                nc.vector.tensor_add(sm[:], sm[:], neg_sb[tci][:])
                mx = spool.tile([128, 1], fp32, tag="mx")
                nc.vector.reduce_max(mx[:], sm[:], mybir.AxisListType.X)
                nmx = spool.tile([128, 1], fp32, tag="nmx")
                nc.vector.tensor_scalar_mul(nmx[:], mx[:], -1.0)
                e_t = spool.tile([128, NOFF], fp32, tag="e")
                nc.scalar.activation(e_t[:], sm[:], AF.Exp, bias=nmx[:, 0:1])
                ev = spool.tile([128, NOFF], fp32, tag="ev")
                nc.vector.tensor_mul(ev[:], e_t[:], valid_sb[tci][:])
                ssum = spool.tile([128, 1], fp32, tag="ssum")
                nc.vector.reduce_sum(ssum[:], ev[:], mybir.AxisListType.X)
                den = spool.tile([128, 1], fp32, tag="den")
                nc.vector.tensor_scalar_add(den[:], ssum[:], 1e-30)
                rden = spool.tile([128, 1], fp32, tag="rden")
                nc.vector.reciprocal(rden[:], den[:])
                cw_t = spool.tile([128, NOFF], fp32, tag="cw")
                nc.vector.tensor_scalar_mul(cw_t[:], ev[:], rden[:, 0:1])
                sc = spool.tile([128, 1], fp32, tag="sc")
                nc.vector.tensor_mul(sc[:], ssum[:], rden[:])

                # router: softmax over experts, scaled by sum_cw
                lg = spool.tile([128, K], fp32, tag="lg")
                nc.vector.tensor_add(lg[:], lg_ps[:], br_sb[:])
                wmx = spool.tile([128, 1], fp32, tag="wmx")
                nc.vector.reduce_max(wmx[:], lg[:], mybir.AxisListType.X)
                nwmx = spool.tile([128, 1], fp32, tag="nwmx")
                nc.vector.tensor_scalar_mul(nwmx[:], wmx[:], -1.0)
                we = spool.tile([128, K], fp32, tag="we")
                nc.scalar.activation(we[:], lg[:], AF.Exp, bias=nwmx[:, 0:1])
                wsum = spool.tile([128, 1], fp32, tag="wsum")
                nc.vector.reduce_sum(wsum[:], we[:], mybir.AxisListType.X)
                rws = spool.tile([128, 1], fp32, tag="rws")
                nc.vector.reciprocal(rws[:], wsum[:])
                wnorm = spool.tile([128, 1], fp32, tag="wnorm")
                nc.vector.tensor_mul(wnorm[:], rws[:], sc[:])
                w_eff = spool.tile([128, K], fp32, tag="weff")
                nc.vector.tensor_scalar_mul(w_eff[:], we[:], wnorm[:, 0:1])

                # transpose cw [128,10] -> [10,128] and w_eff [128,8] -> [8,128]
                cwT_ps = pss.tile([NOFF, 128], fp32, tag="s")
                nc.tensor.transpose(cwT_ps[:], cw_t[:], ident_sb[:])
                nc.scalar.copy(cwT_bf[:, tci * 128:(tci + 1) * 128], cwT_ps[:])
                wT_ps = pss.tile([K, 128], fp32, tag="s")
                nc.tensor.transpose(wT_ps[:], w_eff[:], ident_sb[:])
                nc.scalar.copy(wT_bf[:, tci * 128:(tci + 1) * 128], wT_ps[:])

            # ---- broadcast cw rows / w rows across 128 partitions ----
            # PE operands must start at partition 0/32/64, so first pull the
            # transposed rows down to partition 0 with an SBUF->SBUF DMA.
            cw_rows = ppool.tile([1, NOFF * T], bf16, tag="cw_rows")
            nc.sync.dma_start(cw_rows[:], cwT_bf[:])
            w_rows = ppool.tile([1, K * T], bf16, tag="w_rows")
            nc.sync.dma_start(w_rows[:], wT_bf[:])
            cw_bc = gpool.tile([128, NOFF * 512], bf16, tag="cw_bc")
            for n in range(NOFF):
                bc_ps = psb.tile([128, 512], fp32, tag="m")
                nc.tensor.matmul(bc_ps[:], ones_sb[:],
                                 cw_rows[:, n * T:(n + 1) * T],
                                 start=True, stop=True)
                nc.scalar.copy(cw_bc[:, n * 512:(n + 1) * 512], bc_ps[:])
            w_bc = []
            for k in range(K):
                bc_ps = psb.tile([128, 512], fp32, tag="m")
                nc.tensor.matmul(bc_ps[:], ones_sb[:],
                                 w_rows[:, k * T:(k + 1) * T],
                                 start=True, stop=True)
                wt = gpool.tile([128, 512], bf16, tag=f"w_bc{k}")
                nc.scalar.copy(wt[:], bc_ps[:])
                w_bc.append(wt)

            # ------------- stage D: u/v matmuls + gelu combine -------------
            g_sb = []
            for fc in range(NFC):
                w1b_t = w1pool.tile([128, D], bf16, tag="w1b")
                nc.sync.dma_start(w1b_t[:], w1b[fc])
                w1a_t = w1pool.tile([128, D], bf16, tag="w1a")
                nc.sync.dma_start(w1a_t[:], w1a[fc])

                u_ps = psb.tile([128, 512], fp32, tag="m")
                va_ps = psb.tile([128, 512], fp32, tag="m")
                vb_ps = pss.tile([128, 48], fp32, tag="s")
                for kc in range(NKC):
                    st = (kc == 0)
                    sp = (kc == NKC - 1)
                    lhs_b = w1b_t[:, kc * 128:(kc + 1) * 128]
                    lhs_a = w1a_t[:, kc * 128:(kc + 1) * 128]
                    nc.tensor.matmul(u_ps[:], lhs_b,
                                     h_sb[kc][:, PADL:PADL + 512],
                                     start=st, stop=sp)
                    nc.tensor.matmul(va_ps[:], lhs_a,
                                     h_sb[kc][:, 0:512],
                                     start=st, stop=sp)
                    nc.tensor.matmul(vb_ps[:], lhs_a,
                                     h_sb[kc][:, 496:544],
                                     start=st, stop=sp)

                u_sb = uvpool.tile([128, 512], bf16, tag="u")
                nc.scalar.copy(u_sb[:], u_ps[:])
                v_ev = uvpool.tile([128, TP], bf16, tag="v_ev")
                nc.scalar.copy(v_ev[:, 0:512], va_ps[:])
                nc.scalar.copy(v_ev[:, 512:544], vb_ps[:, 16:48])
                v_od = uvpool.tile([128, TP], bf16, tag="v_od")
                nc.gpsimd.tensor_copy(v_od[:, 0:TP - 1], v_ev[:, 1:TP])

                tmp = bigpool.tile([128, NOFF * 512], bf16, tag="tmp")
                for n, off in enumerate(OFF_ORDER):
                    if off % 2 == 0:
                        src = v_ev[:, PADL + off:PADL + off + 512]
                    else:
                        src = v_od[:, PADL - 1 + off:PADL - 1 + off + 512]
                    nc.vector.tensor_add(
                        tmp[:, n * 512:(n + 1) * 512], src, u_sb[:])
                nc.scalar.activation(tmp[:], tmp[:], AF.Gelu,
                                     bias=b1_sb[:, fc:fc + 1])
                q = q1pool.tile([128, NOFF * 512], bf16, tag="q")
                nc.gpsimd.tensor_mul(q[:], tmp[:], cw_bc[:])

                # pairwise tree-sum of the 10 weighted slices, then w-scale
                t1 = qpool.tile([128, 2560], bf16, tag="t1")
                nc.vector.tensor_add(t1[:], q[:, 0:2560], q[:, 2560:5120])
                t2 = qpool.tile([128, 1024], bf16, tag="t2")
                nc.vector.tensor_add(t2[:], t1[:, 0:1024], t1[:, 1024:2048])
                t3 = qpool.tile([128, 512], bf16, tag="t3")
                nc.vector.tensor_add(t3[:], t2[:, 0:512], t2[:, 512:1024])
                t4 = qpool.tile([128, 512], bf16, tag="t4")
                nc.vector.tensor_add(t4[:], t3[:], t1[:, 2048:2560])
                g_t = gpool.tile([128, 512], bf16, tag=f"g{fc}")
                nc.vector.tensor_mul(g_t[:], t4[:], w_bc[fc // 2][:])
                g_sb.append(g_t)

            # ------------- stage E: delta = (w*g) @ W2 + w_eff @ b2 -------------
            for tci in range(NTC):
                for dh in range(2):
                    d_ps = psb.tile([128, 512], fp32, tag="m")
                    for fc in range(NFC):
                        nc.tensor.matmul(
                            d_ps[:],
                            g_sb[fc][:, tci * 128:(tci + 1) * 128],
                            w2_sb[fc][:, dh * 512:(dh + 1) * 512],
                            start=(fc == 0), stop=False)
                    nc.tensor.matmul(
                        d_ps[:],
                        wT_bf[:, tci * 128:(tci + 1) * 128],
                        b2_sb[:, dh * 512:(dh + 1) * 512],
                        start=False, stop=True)
                    o_sb = opool.tile([128, 512], fp32, tag="o")
                    nc.vector.tensor_copy(o_sb[:], d_ps[:])
                    nc.sync.dma_start(
                        out[tci * 128:(tci + 1) * 128,
                            dh * 512:(dh + 1) * 512], o_sb[:])

    nc.compile()
    return nc


def _prep_shards(h_L, mask_flags, Wr, br, W1, b1, W2, b2):
    """Host-side shard construction (numpy only; cheap vs device work)."""
    f32 = np.float32
    h_L = np.asarray(h_L, f32)
    mask = np.asarray(mask_flags)
    Wr = np.asarray(Wr, f32)
    W1 = np.asarray(W1, f32)
    W2 = np.asarray(W2, f32)
    br = np.asarray(br, f32)
    b1 = np.asarray(b1, f32)
    b2 = np.asarray(b2, f32)

    # shared (replicated) weight blocks
    w1a = np.ascontiguousarray(
        W1[:, :D, :].transpose(1, 0, 2).reshape(D, F)
        .reshape(NKC, 128, NFC, 128).transpose(2, 1, 0, 3)
        .reshape(NFC, 128, D)).astype(BF16)
    w1b = np.ascontiguousarray(
        W1[:, D:, :].transpose(1, 0, 2).reshape(D, F)
        .reshape(NKC, 128, NFC, 128).transpose(2, 1, 0, 3)
        .reshape(NFC, 128, D)).astype(BF16)
    w2 = np.ascontiguousarray(W2.reshape(F, D).reshape(NFC, 128, D)).astype(BF16)
    wr = np.ascontiguousarray(Wr.reshape(NKC, 128, K)).astype(BF16)
    br_bc = np.broadcast_to(br[None, :], (128, K)).copy()
    b1s = np.ascontiguousarray(b1.reshape(F).reshape(NFC, 128).T)
    b2s = b2.astype(BF16)
    identm = np.eye(128, dtype=f32)
    onesm = np.ones((1, 128), dtype=BF16)

    offs = np.array(OFF_ORDER, np.int64)
    in_maps = []
    outs_meta = []
    per_batch = L // (NCORES // B)          # 512 tokens, 4 shards per batch
    for c in range(NCORES):
        b = c // (NCORES // B)
        t0 = (c % (NCORES // B)) * per_batch
        # padded, transposed h slice  [D, TP]
        hpad = np.zeros((TP, D), f32)
        lo = t0 - PADL
        hi = t0 + T + PADL
        slo, shi = max(lo, 0), min(hi, L)
        hpad[slo - lo:shi - lo] = h_L[b, slo:shi]
        hT = np.ascontiguousarray(hpad.T).astype(BF16)          # [D, TP]
        hT = np.ascontiguousarray(hT.reshape(NKC, 128, TP))

        # validity per (token, offset-order)
        tok = t0 + np.arange(T)
        nbr = tok[:, None] + offs[None, :]
        inb = (nbr >= 0) & (nbr < L)
        nbrc = np.clip(nbr, 0, L - 1)
        is_m = (mask[b] == 1)
        val = (inb & is_m[tok][:, None] & (~is_m[nbrc])).astype(f32)
        neg = (val - 1.0) * 1e30
        in_maps.append({
            "hT": hT,
            "w1a": w1a, "w1b": w1b, "w2": w2, "wr": wr,
            "valid": np.ascontiguousarray(val.reshape(NTC, 128, NOFF)),
            "negm": np.ascontiguousarray(neg.reshape(NTC, 128, NOFF)),
            "br_bc": br_bc, "b1s": b1s, "b2s": b2s,
            "ident": identm, "ones": onesm,
        })
        outs_meta.append((b, t0))
    return in_maps, outs_meta


def kernel(**inputs):
    assert int(inputs["range_r"]) == R
    if "nc" not in _CACHE:
        _CACHE["nc"] = _build_graph()
    nc = _CACHE["nc"]
    in_maps, outs_meta = _prep_shards(
        inputs["h_L"], inputs["mask_flags"], inputs["Wr"], inputs["br"],
        inputs["W1"], inputs["b1"], inputs["W2"], inputs["b2"])
    res = run_bass_kernel_spmd(nc, in_maps, core_ids=list(range(NCORES)))
    out = np.zeros((B, L, D), np.float32)
    for c, (b, t0) in enumerate(outs_meta):
        out[b, t0:t0 + T] = res.results[c]["out"]
    return out


# revision 16
# speedup vs baseline: 1.0922x; 1.0922x over previous
"""Trainium2 Bass kernel for nn_AMIPRouterInference (gnn_message_passing).

Strategy
--------
Algebraic restructure of the reference (~515 GFLOP -> ~52 GFLOP):
  * cond @ W1 splits into h_anc @ W1a + h_ctr @ W1b, each computed once per
    token (not once per window pair):  u = h @ W1b, v = h @ W1a.
  * The attention combine over the +-r window commutes with the W2 matmul:
    g = sum_n cw_n * gelu(v[l+off_n] + u[l]);  delta = (w * g) @ W2 + w @ b2.

Sharding: pure data-parallel over the B*L = 4096 tokens -> 512 tokens/core on
8 cores; the +-5 halo is baked into each core's input shard on the host, so no
collectives are needed.

Per-core layout: features-on-partitions (u/v as 16 chunks of [128, tokens]) so
window shifts along tokens are free-axis SBUF slices.  Even/odd phase copies of
v keep the bf16 DVE 2x alignment for shifted adds.
"""

import sys

for _p in ("/opt/trn_rl_repo", "/root/.axon_site/_ro/trn_rl_repo"):
    if _p not in sys.path:
        sys.path.append(_p)

import numpy as np
import ml_dtypes

import concourse.bacc as bacc
import concourse.mybir as mybir
import concourse.tile as tile
from concourse.bass_utils import run_bass_kernel_spmd

BF16 = ml_dtypes.bfloat16

# Problem constants (hardcoded per spec).
B, L, D = 2, 2048, 1024
K, D4, R = 8, 256, 5
NCORES = 8
T = (B * L) // NCORES          # tokens per core = 512
PADL = 16                      # left pad of the per-core token window
TP = T + 2 * PADL              # padded width = 544
NOFF = 2 * R                   # 10 window offsets
F = K * D4                     # 2048 fused expert features
NFC = F // 128                 # 16 feature chunks
NKC = D // 128                 # 8 contraction chunks
NTC = T // 128                 # 4 token tiles per core

# Offset processing order: even offsets first (read from v_even), then odd
# (read from v_odd, which holds v shifted left by one token).  Within each
# phase the SBUF slice starts are even element indices -> 4-byte aligned.
OFF_ORDER = [-4, -2, 2, 4, -5, -3, -1, 1, 3, 5]

_CACHE = {}


def _build_graph():
    fp32 = mybir.dt.float32
    bf16 = mybir.dt.bfloat16

    nc = bacc.Bacc("TRN2", target_bir_lowering=False, debug=False,
                   num_devices=NCORES)

    # ---- DRAM parameters (per-core shards; same shapes on every core) ----
    hT = nc.dram_tensor("hT", [NKC, 128, TP], bf16, kind="ExternalInput")
    w1a = nc.dram_tensor("w1a", [NFC, 128, D], bf16, kind="ExternalInput")
    w1b = nc.dram_tensor("w1b", [NFC, 128, D], bf16, kind="ExternalInput")
    w2 = nc.dram_tensor("w2", [NFC, 128, D], bf16, kind="ExternalInput")
    wr = nc.dram_tensor("wr", [NKC, 128, K], bf16, kind="ExternalInput")
    valid = nc.dram_tensor("valid", [NTC, 128, NOFF], fp32, kind="ExternalInput")
    negm = nc.dram_tensor("negm", [NTC, 128, NOFF], fp32, kind="ExternalInput")
    br_bc = nc.dram_tensor("br_bc", [128, K], fp32, kind="ExternalInput")
    b1s = nc.dram_tensor("b1s", [128, NFC], fp32, kind="ExternalInput")
    b2s = nc.dram_tensor("b2s", [K, D], bf16, kind="ExternalInput")
    ident = nc.dram_tensor("ident", [128, 128], fp32, kind="ExternalInput")
    ones = nc.dram_tensor("ones", [1, 128], bf16, kind="ExternalInput")
    out = nc.dram_tensor("out", [T, D], fp32, kind="ExternalOutput")

    AF = mybir.ActivationFunctionType
    OP = mybir.AluOpType

    with tile.TileContext(nc) as tc:
        with (
            tc.tile_pool(name="const", bufs=1) as cpool,
            tc.tile_pool(name="hpool", bufs=1) as hpool,
            tc.tile_pool(name="w2pool", bufs=1) as w2pool,
            tc.tile_pool(name="w1pool", bufs=3) as w1pool,
            tc.tile_pool(name="small", bufs=3) as spool,
            tc.tile_pool(name="persist", bufs=1) as ppool,
            tc.tile_pool(name="uv", bufs=2) as uvpool,
            tc.tile_pool(name="big", bufs=2) as bigpool,
            tc.tile_pool(name="qbuf", bufs=1) as q1pool,
            tc.tile_pool(name="tbuf", bufs=2) as qpool,
            tc.tile_pool(name="gout", bufs=1) as gpool,
            tc.tile_pool(name="opool", bufs=2) as opool,
            tc.tile_pool(name="dram", bufs=1, space="DRAM") as dpool,
            tc.tile_pool(name="ps_big", bufs=4, space="PSUM") as psb,
            tc.tile_pool(name="ps_small", bufs=4, space="PSUM") as pss,
            # NOTE: all psb tiles share tag "m" (4 banks), all pss tiles share
            # tag "s" (4 banks) -- PSUM has only 8 banks total.
        ):
            # ---------------- constant / input loads ----------------
            h_sb = []
            for kc in range(NKC):
                t = hpool.tile([128, TP], bf16, tag=f"h{kc}")
                nc.sync.dma_start(t[:], hT[kc])
                h_sb.append(t)
            wr_sb = []
            for kc in range(NKC):
                t = cpool.tile([128, K], bf16, tag=f"wr{kc}")
                nc.sync.dma_start(t[:], wr[kc])
                wr_sb.append(t)
            w2_sb = []
            for fc in range(NFC):
                t = w2pool.tile([128, D], bf16, tag=f"w2_{fc}")
                nc.sync.dma_start(t[:], w2[fc])
                w2_sb.append(t)
            ident_sb = cpool.tile([128, 128], fp32, tag="ident")
            nc.sync.dma_start(ident_sb[:], ident[:])
            ones_sb = cpool.tile([1, 128], bf16, tag="ones")
            nc.sync.dma_start(ones_sb[:], ones[:])
            br_sb = cpool.tile([128, K], fp32, tag="br")
            nc.sync.dma_start(br_sb[:], br_bc[:])
            b1_sb = cpool.tile([128, NFC], fp32, tag="b1")
            nc.sync.dma_start(b1_sb[:], b1s[:])
            b2_sb = cpool.tile([K, D], bf16, tag="b2")
            nc.sync.dma_start(b2_sb[:], b2s[:])
            valid_sb, neg_sb = [], []
            for tci in range(NTC):
                tv = cpool.tile([128, NOFF], fp32, tag=f"val{tci}")
                nc.sync.dma_start(tv[:], valid[tci])
                valid_sb.append(tv)
                tn = cpool.tile([128, NOFF], fp32, tag=f"neg{tci}")
                nc.sync.dma_start(tn[:], negm[tci])
                neg_sb.append(tn)

            # persistent transposed score & router weights (bf16)
            cwT_bf = ppool.tile([NOFF, T], bf16, tag="cwT")
            wT_bf = ppool.tile([K, T], bf16, tag="wT")

            # ------------- stage A/B/C: scores, cw, router w -------------
            for tci in range(NTC):
                c0 = PADL + tci * 128
                g_ps = psb.tile([128, 512], fp32, tag="m")
                lg_ps = pss.tile([128, K], fp32, tag="s")
                for kc in range(NKC):
                    st = (kc == 0)
                    sp = (kc == NKC - 1)
                    nc.tensor.matmul(g_ps[:, :138],
                                     h_sb[kc][:, c0:c0 + 128],
                                     h_sb[kc][:, c0 - 5:c0 + 133],
                                     start=st, stop=sp)
                    nc.tensor.matmul(lg_ps[:],
                                     h_sb[kc][:, c0:c0 + 128],
                                     wr_sb[kc][:],
                                     start=st, stop=sp)
                # scores: extract 10 shifted diagonals of the gram tile
                s_t = spool.tile([128, NOFF], fp32, tag="s")
                junk = spool.tile([128, 128], fp32, tag="junk")
                for n, off in enumerate(OFF_ORDER):
                    nc.vector.affine_mul_reduce(
                        junk[:], s_t[:, n:n + 1],
                        g_ps[:, off + 5:off + 5 + 128], ident_sb[:],
                        1.0 / 32.0, 0.0)
                # masked softmax over the offset axis
                sm = spool.tile([128, NOFF], fp32, tag="sm")
                nc.vector.tensor_mul(sm[:], s_t[:], valid_sb[tci][:])
                nc.vector.tensor_add(sm[:], sm[:], neg_sb[tci][:])
                mx = spool.tile([128, 1], fp32, tag="mx")
                nc.vector.reduce_max(mx[:], sm[:], mybir.AxisListType.X)
                nmx = spool.tile([128, 1], fp32, tag="nmx")
                nc.vector.tensor_scalar_mul(nmx[:], mx[:], -1.0)
                e_t = spool.tile([128, NOFF], fp32, tag="e")
                nc.scalar.activation(e_t[:], sm[:], AF.Exp, bias=nmx[:, 0:1])
                ev = spool.tile([128, NOFF], fp32, tag="ev")
                nc.vector.tensor_mul(ev[:], e_t[:], valid_sb[tci][:])
                ssum = spool.tile([128, 1], fp32, tag="ssum")
                nc.vector.reduce_sum(ssum[:], ev[:], mybir.AxisListType.X)
                den = spool.tile([128, 1], fp32, tag="den")
                nc.vector.tensor_scalar_add(den[:], ssum[:], 1e-30)
                rden = spool.tile([128, 1], fp32, tag="rden")
                nc.vector.reciprocal(rden[:], den[:])
                cw_t = spool.tile([128, NOFF], fp32, tag="cw")
                nc.vector.tensor_scalar_mul(cw_t[:], ev[:], rden[:, 0:1])
                sc = spool.tile([128, 1], fp32, tag="sc")
                nc.vector.tensor_mul(sc[:], ssum[:], rden[:])

                # router: softmax over experts, scaled by sum_cw
                lg = spool.tile([128, K], fp32, tag="lg")
                nc.vector.tensor_add(lg[:], lg_ps[:], br_sb[:])
                wmx = spool.tile([128, 1], fp32, tag="wmx")
                nc.vector.reduce_max(wmx[:], lg[:], mybir.AxisListType.X)
                nwmx = spool.tile([128, 1], fp32, tag="nwmx")
                nc.vector.tensor_scalar_mul(nwmx[:], wmx[:], -1.0)
                we = spool.tile([128, K], fp32, tag="we")
                nc.scalar.activation(we[:], lg[:], AF.Exp, bias=nwmx[:, 0:1])
                wsum = spool.tile([128, 1], fp32, tag="wsum")
                nc.vector.reduce_sum(wsum[:], we[:], mybir.AxisListType.X)
                rws = spool.tile([128, 1], fp32, tag="rws")
                nc.vector.reciprocal(rws[:], wsum[:])
                wnorm = spool.tile([128, 1], fp32, tag="wnorm")
                nc.vector.tensor_mul(wnorm[:], rws[:], sc[:])
                w_eff = spool.tile([128, K], fp32, tag="weff")
                nc.vector.tensor_scalar_mul(w_eff[:], we[:], wnorm[:, 0:1])

                # transpose cw [128,10] -> [10,128] and w_eff [128,8] -> [8,128]
                cwT_ps = pss.tile([NOFF, 128], fp32, tag="s")
                nc.tensor.transpose(cwT_ps[:], cw_t[:], ident_sb[:])
                nc.scalar.copy(cwT_bf[:, tci * 128:(tci + 1) * 128], cwT_ps[:])
                wT_ps = pss.tile([K, 128], fp32, tag="s")
                nc.tensor.transpose(wT_ps[:], w_eff[:], ident_sb[:])
                nc.scalar.copy(wT_bf[:, tci * 128:(tci + 1) * 128], wT_ps[:])

            # ------------- stage D: u/v matmuls + gelu combine -------------
            g_sb = [None] * NFC

            def stage_d(fc):
                w1b_t = w1pool.tile([128, D], bf16, tag="w1b")
                nc.sync.dma_start(w1b_t[:], w1b[fc])
                w1a_t = w1pool.tile([128, D], bf16, tag="w1a")
                nc.sync.dma_start(w1a_t[:], w1a[fc])

                u_ps = psb.tile([128, 512], fp32, tag="m")
                va_ps = psb.tile([128, 512], fp32, tag="m")
                vb_ps = pss.tile([128, 48], fp32, tag="s")
                for kc in range(NKC):
                    st = (kc == 0)
                    sp = (kc == NKC - 1)
                    lhs_b = w1b_t[:, kc * 128:(kc + 1) * 128]
                    lhs_a = w1a_t[:, kc * 128:(kc + 1) * 128]
                    nc.tensor.matmul(u_ps[:], lhs_b,
                                     h_sb[kc][:, PADL:PADL + 512],
                                     start=st, stop=sp)
                    nc.tensor.matmul(va_ps[:], lhs_a,
                                     h_sb[kc][:, 0:512],
                                     start=st, stop=sp)
                    nc.tensor.matmul(vb_ps[:], lhs_a,
                                     h_sb[kc][:, 496:544],
                                     start=st, stop=sp)

                u_sb = uvpool.tile([128, 512], bf16, tag="u")
                nc.scalar.copy(u_sb[:], u_ps[:])
                v_ev = uvpool.tile([128, TP], bf16, tag="v_ev")
                nc.scalar.copy(v_ev[:, 0:512], va_ps[:])
                nc.scalar.copy(v_ev[:, 512:544], vb_ps[:, 16:48])
                v_od = uvpool.tile([128, TP], bf16, tag="v_od")
                nc.scalar.copy(v_od[:, 0:TP - 1], v_ev[:, 1:TP])

                tmp = bigpool.tile([128, NOFF * 512], bf16, tag="tmp")
                for n, off in enumerate(OFF_ORDER):
                    if off % 2 == 0:
                        src = v_ev[:, PADL + off:PADL + off + 512]
                    else:
                        src = v_od[:, PADL - 1 + off:PADL - 1 + off + 512]
                    nc.vector.tensor_add(
                        tmp[:, n * 512:(n + 1) * 512], src, u_sb[:])
                nc.scalar.activation(tmp[:], tmp[:], AF.Gelu,
                                     bias=b1_sb[:, fc:fc + 1])
                q = q1pool.tile([128, NOFF * 512], bf16, tag="q")
                nc.gpsimd.tensor_mul(q[:, 0:2560], tmp[:, 0:2560],
                                     cw_bc[:, 0:2560])
                nc.vector.tensor_mul(q[:, 2560:5120], tmp[:, 2560:5120],
                                     cw_bc[:, 2560:5120])

                # pairwise tree-sum of the 10 weighted slices, then w-scale
                t1 = qpool.tile([128, 2560], bf16, tag="t1")
                nc.vector.tensor_add(t1[:], q[:, 0:2560], q[:, 2560:5120])
                t2 = qpool.tile([128, 1024], bf16, tag="t2")
                nc.vector.tensor_add(t2[:], t1[:, 0:1024], t1[:, 1024:2048])
                t3 = qpool.tile([128, 512], bf16, tag="t3")
                nc.vector.tensor_add(t3[:], t2[:, 0:512], t2[:, 512:1024])
                t4 = qpool.tile([128, 512], bf16, tag="t4")
                nc.vector.tensor_add(t4[:], t3[:], t1[:, 2048:2560])
                g_t = gpool.tile([128, 512], bf16, tag=f"g{fc}")
                nc.vector.tensor_mul(g_t[:], t4[:],
                     w_bc_all[:, (fc // 2) * 512:(fc // 2) * 512 + 512])
                g_sb[fc] = g_t

            # ---- broadcast cw rows / w rows across 128 partitions ----
            # Round-trip through DRAM, then one partition-broadcast DMA per
            # target; no TensorE involvement, so PE's in-order queue never
            # blocks on the softmax chain.
            cw_dram = dpool.tile([1, NOFF * T], bf16, tag="cw_dram")
            nc.sync.dma_start(cw_dram[:], cwT_bf[:])
            w_dram = dpool.tile([1, K * T], bf16, tag="w_dram")
            nc.sync.dma_start(w_dram[:], wT_bf[:])
            cw_bc = gpool.tile([128, NOFF * 512], bf16, tag="cw_bc")
            nc.sync.dma_start(cw_bc[:], cw_dram[:].partition_broadcast(128))
            w_bc_all = gpool.tile([128, K * 512], bf16, tag="w_bc_all")
            nc.sync.dma_start(w_bc_all[:], w_dram[:].partition_broadcast(128))

            for fc in range(NFC):
                stage_d(fc)

            # ------------- stage E: delta = (w*g) @ W2 + w_eff @ b2 -------------
            for tci in range(NTC):
                for dh in range(2):
                    d_ps = psb.tile([128, 512], fp32, tag="m")
                    for fc in range(NFC):
                        nc.tensor.matmul(
                            d_ps[:],
                            g_sb[fc][:, tci * 128:(tci + 1) * 128],
                            w2_sb[fc][:, dh * 512:(dh + 1) * 512],
                            start=(fc == 0), stop=False)
                    nc.tensor.matmul(
                        d_ps[:],
                        wT_bf[:, tci * 128:(tci + 1) * 128],
                        b2_sb[:, dh * 512:(dh + 1) * 512],
                        start=False, stop=True)
                    o_sb = opool.tile([128, 512], fp32, tag="o")
                    nc.vector.tensor_copy(o_sb[:], d_ps[:])
                    nc.sync.dma_start(
                        out[tci * 128:(tci + 1) * 128,
                            dh * 512:(dh + 1) * 512], o_sb[:])

    nc.compile()
    return nc


def _prep_shards(h_L, mask_flags, Wr, br, W1, b1, W2, b2):
    """Host-side shard construction (numpy only; cheap vs device work)."""
    f32 = np.float32
    h_L = np.asarray(h_L, f32)
    mask = np.asarray(mask_flags)
    Wr = np.asarray(Wr, f32)
    W1 = np.asarray(W1, f32)
    W2 = np.asarray(W2, f32)
    br = np.asarray(br, f32)
    b1 = np.asarray(b1, f32)
    b2 = np.asarray(b2, f32)

    # shared (replicated) weight blocks
    w1a = np.ascontiguousarray(
        W1[:, :D, :].transpose(1, 0, 2).reshape(D, F)
        .reshape(NKC, 128, NFC, 128).transpose(2, 1, 0, 3)
        .reshape(NFC, 128, D)).astype(BF16)
    w1b = np.ascontiguousarray(
        W1[:, D:, :].transpose(1, 0, 2).reshape(D, F)
        .reshape(NKC, 128, NFC, 128).transpose(2, 1, 0, 3)
        .reshape(NFC, 128, D)).astype(BF16)
    w2 = np.ascontiguousarray(W2.reshape(F, D).reshape(NFC, 128, D)).astype(BF16)
    wr = np.ascontiguousarray(Wr.reshape(NKC, 128, K)).astype(BF16)
    br_bc = np.broadcast_to(br[None, :], (128, K)).copy()
    b1s = np.ascontiguousarray(b1.reshape(F).reshape(NFC, 128).T)
    b2s = b2.astype(BF16)
    identm = np.eye(128, dtype=f32)
    onesm = np.ones((1, 128), dtype=BF16)

    offs = np.array(OFF_ORDER, np.int64)
    in_maps = []
    outs_meta = []
    per_batch = L // (NCORES // B)          # 512 tokens, 4 shards per batch
    for c in range(NCORES):
        b = c // (NCORES // B)
        t0 = (c % (NCORES // B)) * per_batch
        # padded, transposed h slice  [D, TP]
        hpad = np.zeros((TP, D), f32)
        lo = t0 - PADL
        hi = t0 + T + PADL
        slo, shi = max(lo, 0), min(hi, L)
        hpad[slo - lo:shi - lo] = h_L[b, slo:shi]
        hT = np.ascontiguousarray(hpad.T).astype(BF16)          # [D, TP]
        hT = np.ascontiguousarray(hT.reshape(NKC, 128, TP))

        # validity per (token, offset-order)
        tok = t0 + np.arange(T)
        nbr = tok[:, None] + offs[None, :]
        inb = (nbr >= 0) & (nbr < L)
        nbrc = np.clip(nbr, 0, L - 1)
        is_m = (mask[b] == 1)
        val = (inb & is_m[tok][:, None] & (~is_m[nbrc])).astype(f32)
        neg = (val - 1.0) * 1e30
        in_maps.append({
            "hT": hT,
            "w1a": w1a, "w1b": w1b, "w2": w2, "wr": wr,
            "valid": np.ascontiguousarray(val.reshape(NTC, 128, NOFF)),
            "negm": np.ascontiguousarray(neg.reshape(NTC, 128, NOFF)),
            "br_bc": br_bc, "b1s": b1s, "b2s": b2s,
            "ident": identm, "ones": onesm,
        })
        outs_meta.append((b, t0))
    return in_maps, outs_meta


def kernel(**inputs):
    assert int(inputs["range_r"]) == R
    if "nc" not in _CACHE:
        _CACHE["nc"] = _build_graph()
    nc = _CACHE["nc"]
    in_maps, outs_meta = _prep_shards(
        inputs["h_L"], inputs["mask_flags"], inputs["Wr"], inputs["br"],
        inputs["W1"], inputs["b1"], inputs["W2"], inputs["b2"])
    res = run_bass_kernel_spmd(nc, in_maps, core_ids=list(range(NCORES)))
    out = np.zeros((B, L, D), np.float32)
    for c, (b, t0) in enumerate(outs_meta):
        out[b, t0:t0 + T] = res.results[c]["out"]
    return out


# revision 17
# speedup vs baseline: 1.3713x; 1.2556x over previous
"""Trainium2 Bass kernel for nn_AMIPRouterInference (gnn_message_passing).

Strategy
--------
Algebraic restructure of the reference (~515 GFLOP -> ~52 GFLOP):
  * cond @ W1 splits into h_anc @ W1a + h_ctr @ W1b, each computed once per
    token (not once per window pair):  u = h @ W1b, v = h @ W1a.
  * The attention combine over the +-r window commutes with the W2 matmul:
    g = sum_n cw_n * gelu(v[l+off_n] + u[l]);  delta = (w * g) @ W2 + w @ b2.

Sharding: pure data-parallel over the B*L = 4096 tokens -> 512 tokens/core on
8 cores; the +-5 halo is baked into each core's input shard on the host, so no
collectives are needed.

Per-core layout: features-on-partitions (u/v as 16 chunks of [128, tokens]) so
window shifts along tokens are free-axis SBUF slices.  Even/odd phase copies of
v keep the bf16 DVE 2x alignment for shifted adds.
"""

import sys

for _p in ("/opt/trn_rl_repo", "/root/.axon_site/_ro/trn_rl_repo"):
    if _p not in sys.path:
        sys.path.append(_p)

import numpy as np
import ml_dtypes

import bass_rust
import concourse.bacc as bacc
import concourse.mybir as mybir
import concourse.tile as tile
from concourse.bass_utils import run_bass_kernel_spmd

BF16 = ml_dtypes.bfloat16

# Problem constants (hardcoded per spec).
B, L, D = 2, 2048, 1024
K, D4, R = 8, 256, 5
NCORES = 8
T = (B * L) // NCORES          # tokens per core = 512
PADL = 16                      # left pad of the per-core token window
TP = T + 2 * PADL              # padded width = 544
NOFF = 2 * R                   # 10 window offsets
F = K * D4                     # 2048 fused expert features
NFC = F // 128                 # 16 feature chunks
NKC = D // 128                 # 8 contraction chunks
NTC = T // 128                 # 4 token tiles per core

# Offset processing order: even offsets first (read from v_even), then odd
# (read from v_odd, which holds v shifted left by one token).  Within each
# phase the SBUF slice starts are even element indices -> 4-byte aligned.
OFF_ORDER = [-4, -2, 2, 4, -5, -3, -1, 1, 3, 5]

_CACHE = {}


def _build_graph():
    fp32 = mybir.dt.float32
    bf16 = mybir.dt.bfloat16

    nc = bacc.Bacc("TRN2", target_bir_lowering=False, debug=False,
                   num_devices=NCORES)

    # ---- DRAM parameters (per-core shards; same shapes on every core) ----
    hT = nc.dram_tensor("hT", [NKC, 128, TP], bf16, kind="ExternalInput")
    w1a = nc.dram_tensor("w1a", [NFC, 128, D], bf16, kind="ExternalInput")
    w1b = nc.dram_tensor("w1b", [NFC, 128, D], bf16, kind="ExternalInput")
    w2 = nc.dram_tensor("w2", [NFC, 128, D], bf16, kind="ExternalInput")
    wr = nc.dram_tensor("wr", [NKC, 128, K], bf16, kind="ExternalInput")
    valid = nc.dram_tensor("valid", [NTC, 128, NOFF], fp32, kind="ExternalInput")
    negm = nc.dram_tensor("negm", [NTC, 128, NOFF], fp32, kind="ExternalInput")
    br_bc = nc.dram_tensor("br_bc", [128, K], fp32, kind="ExternalInput")
    b1s = nc.dram_tensor("b1s", [128, NFC], fp32, kind="ExternalInput")
    b2s = nc.dram_tensor("b2s", [K, D], bf16, kind="ExternalInput")
    ident = nc.dram_tensor("ident", [128, 128], fp32, kind="ExternalInput")
    ones = nc.dram_tensor("ones", [1, 128], bf16, kind="ExternalInput")
    out = nc.dram_tensor("out", [T, D], fp32, kind="ExternalOutput")

    AF = mybir.ActivationFunctionType
    OP = mybir.AluOpType

    with tile.TileContext(nc) as tc:
        with (
            tc.tile_pool(name="const", bufs=1) as cpool,
            tc.tile_pool(name="hpool", bufs=1) as hpool,
            tc.tile_pool(name="w2pool", bufs=1) as w2pool,
            tc.tile_pool(name="w1pool", bufs=3) as w1pool,
            tc.tile_pool(name="small", bufs=3) as spool,
            tc.tile_pool(name="persist", bufs=1) as ppool,
            tc.tile_pool(name="uv", bufs=2) as uvpool,
            tc.tile_pool(name="big", bufs=2) as bigpool,
            tc.tile_pool(name="qbuf", bufs=2) as q1pool,
            tc.tile_pool(name="tbuf", bufs=2) as qpool,
            tc.tile_pool(name="gout", bufs=1) as gpool,
            tc.tile_pool(name="opool", bufs=2) as opool,
            tc.tile_pool(name="dram", bufs=1, space="DRAM") as dpool,
            tc.tile_pool(name="ps_big", bufs=4, space="PSUM") as psb,
            tc.tile_pool(name="ps_small", bufs=4, space="PSUM") as pss,
            # NOTE: all psb tiles share tag "m" (4 banks), all pss tiles share
            # tag "s" (4 banks) -- PSUM has only 8 banks total.
        ):
            # ---------------- constant / input loads ----------------
            h_sb = []
            for kc in range(NKC):
                t = hpool.tile([128, TP], bf16, tag=f"h{kc}")
                nc.sync.dma_start(t[:], hT[kc])
                h_sb.append(t)
            wr_sb = []
            for kc in range(NKC):
                t = cpool.tile([128, K], bf16, tag=f"wr{kc}")
                nc.sync.dma_start(t[:], wr[kc])
                wr_sb.append(t)
            w2_sb = []
            for fc in range(NFC):
                t = w2pool.tile([128, D], bf16, tag=f"w2_{fc}")
                nc.sync.dma_start(t[:], w2[fc])
                w2_sb.append(t)
            ident_sb = cpool.tile([128, 128], fp32, tag="ident")
            nc.sync.dma_start(ident_sb[:], ident[:])
            ones_sb = cpool.tile([1, 128], bf16, tag="ones")
            nc.sync.dma_start(ones_sb[:], ones[:])
            br_sb = cpool.tile([128, K], fp32, tag="br")
            nc.sync.dma_start(br_sb[:], br_bc[:])
            b1_sb = cpool.tile([128, NFC], fp32, tag="b1")
            nc.sync.dma_start(b1_sb[:], b1s[:])
            b2_sb = cpool.tile([K, D], bf16, tag="b2")
            nc.sync.dma_start(b2_sb[:], b2s[:])
            valid_sb, neg_sb = [], []
            for tci in range(NTC):
                tv = cpool.tile([128, NOFF], fp32, tag=f"val{tci}")
                nc.sync.dma_start(tv[:], valid[tci])
                valid_sb.append(tv)
                tn = cpool.tile([128, NOFF], fp32, tag=f"neg{tci}")
                nc.sync.dma_start(tn[:], negm[tci])
                neg_sb.append(tn)

            # persistent transposed score & router weights (bf16)
            cwT_bf = ppool.tile([NOFF, T], bf16, tag="cwT")
            wT_bf = ppool.tile([K, T], bf16, tag="wT")

            # ------------- stage A/B/C: scores, cw, router w -------------
            for tci in range(NTC):
                c0 = PADL + tci * 128
                g_ps = psb.tile([128, 512], fp32, tag="m")
                lg_ps = pss.tile([128, K], fp32, tag="s")
                for kc in range(NKC):
                    st = (kc == 0)
                    sp = (kc == NKC - 1)
                    nc.tensor.matmul(g_ps[:, :138],
                                     h_sb[kc][:, c0:c0 + 128],
                                     h_sb[kc][:, c0 - 5:c0 + 133],
                                     start=st, stop=sp)
                    nc.tensor.matmul(lg_ps[:],
                                     h_sb[kc][:, c0:c0 + 128],
                                     wr_sb[kc][:],
                                     start=st, stop=sp)
                # scores: extract 10 shifted diagonals of the gram tile
                s_t = spool.tile([128, NOFF], fp32, tag="s")
                junk = spool.tile([128, 128], fp32, tag="junk")
                for n, off in enumerate(OFF_ORDER):
                    nc.vector.affine_mul_reduce(
                        junk[:], s_t[:, n:n + 1],
                        g_ps[:, off + 5:off + 5 + 128], ident_sb[:],
                        1.0 / 32.0, 0.0)
                # masked softmax over the offset axis
                sm = spool.tile([128, NOFF], fp32, tag="sm")
                nc.vector.tensor_mul(sm[:], s_t[:], valid_sb[tci][:])
                nc.vector.tensor_add(sm[:], sm[:], neg_sb[tci][:])
                mx = spool.tile([128, 1], fp32, tag="mx")
                nc.vector.reduce_max(mx[:], sm[:], mybir.AxisListType.X)
                nmx = spool.tile([128, 1], fp32, tag="nmx")
                nc.vector.tensor_scalar_mul(nmx[:], mx[:], -1.0)
                e_t = spool.tile([128, NOFF], fp32, tag="e")
                nc.scalar.activation(e_t[:], sm[:], AF.Exp, bias=nmx[:, 0:1])
                ev = spool.tile([128, NOFF], fp32, tag="ev")
                nc.vector.tensor_mul(ev[:], e_t[:], valid_sb[tci][:])
                ssum = spool.tile([128, 1], fp32, tag="ssum")
                nc.vector.reduce_sum(ssum[:], ev[:], mybir.AxisListType.X)
                den = spool.tile([128, 1], fp32, tag="den")
                nc.vector.tensor_scalar_add(den[:], ssum[:], 1e-30)
                rden = spool.tile([128, 1], fp32, tag="rden")
                nc.vector.reciprocal(rden[:], den[:])
                cw_t = spool.tile([128, NOFF], fp32, tag="cw")
                nc.vector.tensor_scalar_mul(cw_t[:], ev[:], rden[:, 0:1])
                sc = spool.tile([128, 1], fp32, tag="sc")
                nc.vector.tensor_mul(sc[:], ssum[:], rden[:])

                # router: softmax over experts, scaled by sum_cw
                lg = spool.tile([128, K], fp32, tag="lg")
                nc.vector.tensor_add(lg[:], lg_ps[:], br_sb[:])
                wmx = spool.tile([128, 1], fp32, tag="wmx")
                nc.vector.reduce_max(wmx[:], lg[:], mybir.AxisListType.X)
                nwmx = spool.tile([128, 1], fp32, tag="nwmx")
                nc.vector.tensor_scalar_mul(nwmx[:], wmx[:], -1.0)
                we = spool.tile([128, K], fp32, tag="we")
                nc.scalar.activation(we[:], lg[:], AF.Exp, bias=nwmx[:, 0:1])
                wsum = spool.tile([128, 1], fp32, tag="wsum")
                nc.vector.reduce_sum(wsum[:], we[:], mybir.AxisListType.X)
                rws = spool.tile([128, 1], fp32, tag="rws")
                nc.vector.reciprocal(rws[:], wsum[:])
                wnorm = spool.tile([128, 1], fp32, tag="wnorm")
                nc.vector.tensor_mul(wnorm[:], rws[:], sc[:])
                w_eff = spool.tile([128, K], fp32, tag="weff")
                nc.vector.tensor_scalar_mul(w_eff[:], we[:], wnorm[:, 0:1])

                # transpose cw [128,10] -> [10,128] and w_eff [128,8] -> [8,128]
                cwT_ps = pss.tile([NOFF, 128], fp32, tag="s")
                nc.tensor.transpose(cwT_ps[:], cw_t[:], ident_sb[:])
                nc.scalar.copy(cwT_bf[:, tci * 128:(tci + 1) * 128], cwT_ps[:])
                wT_ps = pss.tile([K, 128], fp32, tag="s")
                nc.tensor.transpose(wT_ps[:], w_eff[:], ident_sb[:])
                nc.scalar.copy(wT_bf[:, tci * 128:(tci + 1) * 128], wT_ps[:])

            # ------------- stage D: u/v matmuls + gelu combine -------------
            g_sb = [None] * NFC

            def stage_d(fc):
                w1b_t = w1pool.tile([128, D], bf16, tag="w1b")
                nc.sync.dma_start(w1b_t[:], w1b[fc])
                w1a_t = w1pool.tile([128, D], bf16, tag="w1a")
                nc.sync.dma_start(w1a_t[:], w1a[fc])

                u_ps = psb.tile([128, 512], fp32, tag="m")
                va_ps = psb.tile([128, 512], fp32, tag="m")
                vb_ps = pss.tile([128, 48], fp32, tag="s")
                for kc in range(NKC):
                    st = (kc == 0)
                    sp = (kc == NKC - 1)
                    lhs_b = w1b_t[:, kc * 128:(kc + 1) * 128]
                    lhs_a = w1a_t[:, kc * 128:(kc + 1) * 128]
                    nc.tensor.matmul(u_ps[:], lhs_b,
                                     h_sb[kc][:, PADL:PADL + 512],
                                     start=st, stop=sp)
                    nc.tensor.matmul(va_ps[:], lhs_a,
                                     h_sb[kc][:, 0:512],
                                     start=st, stop=sp)
                    nc.tensor.matmul(vb_ps[:], lhs_a,
                                     h_sb[kc][:, 496:544],
                                     start=st, stop=sp)

                u_sb = uvpool.tile([128, 512], bf16, tag="u")
                nc.scalar.copy(u_sb[:], u_ps[:])
                v_ev = uvpool.tile([128, TP], bf16, tag="v_ev")
                nc.scalar.copy(v_ev[:, 0:512], va_ps[:])
                nc.scalar.copy(v_ev[:, 512:544], vb_ps[:, 16:48])
                v_od = uvpool.tile([128, TP], bf16, tag="v_od")
                nc.sync.dma_start(v_od[:, 0:TP - 1], v_ev[:, 1:TP])

                tmp = bigpool.tile([128, NOFF * 512], bf16, tag="tmp")
                # batched shifted adds: one op per phase-contiguous offset
                # group, via overlapping window APs (middle dim step 2) and a
                # step-0 broadcast AP on u.
                for vt, start, cnt, lo in (
                        (v_ev, PADL - 4, 2, 0),       # offs -4, -2
                        (v_ev, PADL + 2, 2, 1024),    # offs  2,  4
                        (v_od, PADL - 6, 3, 2048),    # offs -5, -3, -1
                        (v_od, PADL + 0, 3, 3584)):   # offs  1,  3,  5
                    src_ap = bass_rust.AP(
                        tensor=vt[:].tensor, offset=start,
                        ap=[[TP, 128], [2, cnt], [1, 512]])
                    u_ap = bass_rust.AP(
                        tensor=u_sb[:].tensor, offset=0,
                        ap=[[512, 128], [0, cnt], [1, 512]])
                    nc.vector.tensor_add(
                        tmp[:, lo:lo + cnt * 512], src_ap, u_ap)
                nc.scalar.activation(tmp[:], tmp[:], AF.Gelu,
                                     bias=b1_sb[:, fc:fc + 1])
                q = q1pool.tile([128, NOFF * 512], bf16, tag="q")
                nc.gpsimd.tensor_mul(q[:, 0:1024], tmp[:, 0:1024],
                                     cw_bc[:, 0:1024])
                nc.vector.tensor_mul(q[:, 1024:5120], tmp[:, 1024:5120],
                                     cw_bc[:, 1024:5120])

                # pairwise tree-sum of the 10 weighted slices, then w-scale
                t1 = qpool.tile([128, 2560], bf16, tag="t1")
                nc.vector.tensor_add(t1[:], q[:, 0:2560], q[:, 2560:5120])
                t2 = qpool.tile([128, 1024], bf16, tag="t2")
                nc.vector.tensor_add(t2[:], t1[:, 0:1024], t1[:, 1024:2048])
                t3 = qpool.tile([128, 512], bf16, tag="t3")
                nc.vector.tensor_add(t3[:], t2[:, 0:512], t2[:, 512:1024])
                t4 = qpool.tile([128, 512], bf16, tag="t4")
                nc.vector.tensor_add(t4[:], t3[:], t1[:, 2048:2560])
                g_t = gpool.tile([128, 512], bf16, tag=f"g{fc}")
                nc.gpsimd.tensor_mul(
                    g_t[:], t4[:],
                    w_bc_all[:, (fc // 2) * 512:(fc // 2) * 512 + 512])
                g_sb[fc] = g_t

            # ---- broadcast cw rows / w rows across 128 partitions ----
            # Round-trip through DRAM, then one partition-broadcast DMA per
            # target; no TensorE involvement, so PE's in-order queue never
            # blocks on the softmax chain.
            cw_dram = dpool.tile([1, NOFF * T], bf16, tag="cw_dram")
            nc.sync.dma_start(cw_dram[:], cwT_bf[:])
            w_dram = dpool.tile([1, K * T], bf16, tag="w_dram")
            nc.sync.dma_start(w_dram[:], wT_bf[:])
            cw_bc = gpool.tile([128, NOFF * 512], bf16, tag="cw_bc")
            nc.sync.dma_start(cw_bc[:], cw_dram[:].partition_broadcast(128))
            w_bc_all = gpool.tile([128, K * 512], bf16, tag="w_bc_all")
            nc.sync.dma_start(w_bc_all[:], w_dram[:].partition_broadcast(128))

            for fc in range(NFC):
                stage_d(fc)

            # ------------- stage E: delta = (w*g) @ W2 + w_eff @ b2 -------------
            for tci in range(NTC):
                for dh in range(2):
                    d_ps = psb.tile([128, 512], fp32, tag="m")
                    for fc in range(NFC):
                        nc.tensor.matmul(
                            d_ps[:],
                            g_sb[fc][:, tci * 128:(tci + 1) * 128],
                            w2_sb[fc][:, dh * 512:(dh + 1) * 512],
                            start=(fc == 0), stop=False)
                    nc.tensor.matmul(
                        d_ps[:],
                        wT_bf[:, tci * 128:(tci + 1) * 128],
                        b2_sb[:, dh * 512:(dh + 1) * 512],
                        start=False, stop=True)
                    o_sb = opool.tile([128, 512], fp32, tag="o")
                    nc.scalar.copy(o_sb[:], d_ps[:])
                    nc.sync.dma_start(
                        out[tci * 128:(tci + 1) * 128,
                            dh * 512:(dh + 1) * 512], o_sb[:])

    nc.compile()
    return nc


def _prep_shards(h_L, mask_flags, Wr, br, W1, b1, W2, b2):
    """Host-side shard construction (numpy only; cheap vs device work)."""
    f32 = np.float32
    h_L = np.asarray(h_L, f32)
    mask = np.asarray(mask_flags)
    Wr = np.asarray(Wr, f32)
    W1 = np.asarray(W1, f32)
    W2 = np.asarray(W2, f32)
    br = np.asarray(br, f32)
    b1 = np.asarray(b1, f32)
    b2 = np.asarray(b2, f32)

    # shared (replicated) weight blocks
    w1a = np.ascontiguousarray(
        W1[:, :D, :].transpose(1, 0, 2).reshape(D, F)
        .reshape(NKC, 128, NFC, 128).transpose(2, 1, 0, 3)
        .reshape(NFC, 128, D)).astype(BF16)
    w1b = np.ascontiguousarray(
        W1[:, D:, :].transpose(1, 0, 2).reshape(D, F)
        .reshape(NKC, 128, NFC, 128).transpose(2, 1, 0, 3)
        .reshape(NFC, 128, D)).astype(BF16)
    w2 = np.ascontiguousarray(W2.reshape(F, D).reshape(NFC, 128, D)).astype(BF16)
    wr = np.ascontiguousarray(Wr.reshape(NKC, 128, K)).astype(BF16)
    br_bc = np.broadcast_to(br[None, :], (128, K)).copy()
    b1s = np.ascontiguousarray(b1.reshape(F).reshape(NFC, 128).T)
    b2s = b2.astype(BF16)
    identm = np.eye(128, dtype=f32)
    onesm = np.ones((1, 128), dtype=BF16)

    offs = np.array(OFF_ORDER, np.int64)
    in_maps = []
    outs_meta = []
    per_batch = L // (NCORES // B)          # 512 tokens, 4 shards per batch
    for c in range(NCORES):
        b = c // (NCORES // B)
        t0 = (c % (NCORES // B)) * per_batch
        # padded, transposed h slice  [D, TP]
        hpad = np.zeros((TP, D), f32)
        lo = t0 - PADL
        hi = t0 + T + PADL
        slo, shi = max(lo, 0), min(hi, L)
        hpad[slo - lo:shi - lo] = h_L[b, slo:shi]
        hT = np.ascontiguousarray(hpad.T).astype(BF16)          # [D, TP]
        hT = np.ascontiguousarray(hT.reshape(NKC, 128, TP))

        # validity per (token, offset-order)
        tok = t0 + np.arange(T)
        nbr = tok[:, None] + offs[None, :]
        inb = (nbr >= 0) & (nbr < L)
        nbrc = np.clip(nbr, 0, L - 1)
        is_m = (mask[b] == 1)
        val = (inb & is_m[tok][:, None] & (~is_m[nbrc])).astype(f32)
        neg = (val - 1.0) * 1e30
        in_maps.append({
            "hT": hT,
            "w1a": w1a, "w1b": w1b, "w2": w2, "wr": wr,
            "valid": np.ascontiguousarray(val.reshape(NTC, 128, NOFF)),
            "negm": np.ascontiguousarray(neg.reshape(NTC, 128, NOFF)),
            "br_bc": br_bc, "b1s": b1s, "b2s": b2s,
            "ident": identm, "ones": onesm,
        })
        outs_meta.append((b, t0))
    return in_maps, outs_meta


def kernel(**inputs):
    assert int(inputs["range_r"]) == R
    if "nc" not in _CACHE:
        _CACHE["nc"] = _build_graph()
    nc = _CACHE["nc"]
    in_maps, outs_meta = _prep_shards(
        inputs["h_L"], inputs["mask_flags"], inputs["Wr"], inputs["br"],
        inputs["W1"], inputs["b1"], inputs["W2"], inputs["b2"])
    res = run_bass_kernel_spmd(nc, in_maps, core_ids=list(range(NCORES)))
    out = np.zeros((B, L, D), np.float32)
    for c, (b, t0) in enumerate(outs_meta):
        out[b, t0:t0 + T] = res.results[c]["out"]
    return out
